# revision 13
# baseline (speedup 1.0000x reference)
"""Mamba-130m forward (B=64, T=8, 24 layers, tied LM head) on 8 Trainium2 cores.

Strategy: data-parallel over batch (8 batches/core, 64 tokens/core), full layer
stack per core, token-sharded LM head (each core computes the full vocab for its
own tokens).  No collectives.  Weights are pre-transposed / folded on the host
and replicated to every core in bf16; all matmuls run bf16 with fp32 PSUM
accumulation.  The selective scan runs on the Vector engine with
tensor_tensor_scan, exploiting A[ch, n] = -n (verified at runtime) so the decay
factors are powers r^n of r = exp(-dt).
"""

import os
import sys
import tempfile

# The libneuronxla NEFF cache hashes the HLO wrapper but not the embedded BIR
# (frontend attributes), so two different Bass programs collide on the same
# cache key.  Use a fresh per-process cache dir to avoid stale-NEFF reuse.
if "NEURON_COMPILE_CACHE_URL" not in os.environ:
    os.environ["NEURON_COMPILE_CACHE_URL"] = tempfile.mkdtemp(
        prefix="neuron-cache-")

sys.path.insert(0, "/opt/trn_rl_repo")

import numpy as np
import ml_dtypes

import concourse.bass as bass
import concourse.bacc as bacc
import concourse.tile as tile
import concourse.mybir as mybir
from concourse.bass_utils import run_bass_kernel_spmd

F32 = mybir.dt.float32
BF16 = mybir.dt.bfloat16
AF = mybir.ActivationFunctionType
OP = mybir.AluOpType
# debug: CoreSim has no Silu; MK_SIGMOID=1 swaps in Sigmoid (wrong math, same
# program structure) for sim-vs-hw divergence checks.
AF_SILU = AF.Sigmoid if os.environ.get("MK_SIGMOID", "0") == "1" else AF.Silu

N_LAYERS = int(os.environ.get("MK_LAYERS", "24"))
# Bump on every kernel change: encoded into a dummy input's shape so the HLO
# hash (and the terminal-side NEFF cache key) changes with the program.
KERNEL_VERSION = 4
D_MODEL = 768
D_INNER = 1536
D_STATE = 16
DT_RANK = 48
CONV_K = 4
VOCAB = 50280
EPS = 1e-5
SEQ = 8
B_FULL = 64
NCORES = 8
B_LOC = B_FULL // NCORES      # 8 batches per core
TOK = B_LOC * SEQ             # 64 tokens per core, tok = b*8 + t
NKD = D_MODEL // 128          # 6 d_model chunks
NCH = D_INNER // 128          # 12 d_inner chunks

_cache = {}


def _build_program():
    """Build the SPMD Bass program (identical on all 8 cores)."""
    nc = bacc.Bacc("TRN2", target_bir_lowering=False, debug=False,
                   num_devices=NCORES)

    # ---- DRAM I/O ----
    h0_d = nc.dram_tensor("h0", [TOK, D_MODEL], F32, kind="ExternalInput")
    inT_d = nc.dram_tensor("inT", [N_LAYERS, D_MODEL, 2 * D_INNER], BF16,
                           kind="ExternalInput")
    outT_d = nc.dram_tensor("outT", [N_LAYERS, D_INNER, D_MODEL], BF16,
                            kind="ExternalInput")
    xpT_d = nc.dram_tensor("xpT", [N_LAYERS, D_INNER, 112],
                           BF16, kind="ExternalInput")
    dtT_d = nc.dram_tensor("dtT", [N_LAYERS, DT_RANK, D_INNER], BF16,
                           kind="ExternalInput")
    embT_d = nc.dram_tensor("embT", [D_MODEL, VOCAB], BF16,
                            kind="ExternalInput")
    # per-layer per-chunk scalars: [L, 128, NCH, 8] = conv_w(4), conv_b, dt_b, D, pad
    sm_d = nc.dram_tensor("smalls", [N_LAYERS, 128, NCH, 8], F32,
                          kind="ExternalInput")
    eye_d = nc.dram_tensor("eye", [128, 128], F32, kind="ExternalInput")
    ones_d = nc.dram_tensor("ones1", [1, 128], BF16, kind="ExternalInput")
    nc.dram_tensor("vtag", [1, KERNEL_VERSION * 64 + N_LAYERS], F32,
                   kind="ExternalInput")
    out_d = nc.dram_tensor("logits", [TOK, VOCAB], F32, kind="ExternalOutput")

    from contextlib import ExitStack
    with ExitStack() as ctx:
        tc = ctx.enter_context(tile.TileContext(nc))
        constp = ctx.enter_context(tc.tile_pool(name="const", bufs=1))
        hpool = ctx.enter_context(tc.tile_pool(name="hpool", bufs=2))
        scrp = ctx.enter_context(tc.tile_pool(name="scr", bufs=2))
        tinyp = ctx.enter_context(tc.tile_pool(name="tiny", bufs=4))
        hnfmp = ctx.enter_context(tc.tile_pool(name="hnfm", bufs=14))
        wrowp = ctx.enter_context(tc.tile_pool(name="wrow", bufs=7))
        worowp = ctx.enter_context(tc.tile_pool(name="worow", bufs=14))
        wxpp = ctx.enter_context(tc.tile_pool(name="wxp", bufs=14))
        wdtp = ctx.enter_context(tc.tile_pool(name="wdt", bufs=2))
        wsmp = ctx.enter_context(tc.tile_pool(name="wsm", bufs=2))
        xcp = ctx.enter_context(tc.tile_pool(name="xcp", bufs=14))
        convp = ctx.enter_context(tc.tile_pool(name="convp", bufs=3))
        ktmp = ctx.enter_context(tc.tile_pool(name="ktmp", bufs=4))
        scanp = ctx.enter_context(tc.tile_pool(name="scan", bufs=3))
        bcp = ctx.enter_context(tc.tile_pool(name="bc", bufs=2))
        embp = ctx.enter_context(tc.tile_pool(name="emb", bufs=10))
        lgoutp = ctx.enter_context(tc.tile_pool(name="lgout", bufs=6))
        psA = ctx.enter_context(tc.tile_pool(name="psA", bufs=3, space="PSUM"))
        pbig = ctx.enter_context(tc.tile_pool(name="pbig", bufs=2, space="PSUM"))
        psB = ctx.enter_context(tc.tile_pool(name="psB", bufs=1, space="PSUM"))
        dramp = ctx.enter_context(tc.tile_pool(name="dram", bufs=4, space="DRAM"))
        if True:
            eye = constp.tile([128, 128], F32)
            nc.sync.dma_start(eye[:], eye_d[:, :])
            ones1 = constp.tile([1, 128], BF16)
            nc.sync.dma_start(ones1[:], ones_d[:, :])
            zeps = constp.tile([128, 3], F32)
            nc.vector.memset(zeps[:, 0:1], 0.0)
            nc.vector.memset(zeps[:, 1:2], EPS)
            nc.vector.memset(zeps[:, 2:3], 1.0)

            h = hpool.tile([TOK, D_MODEL], F32, tag="h")
            nc.sync.dma_start(h[:], h0_d[:, :])

            def rmsnorm_scale(h_in):
                """returns s [TOK,1] f32 with s = rsqrt(mean(h^2)+eps)"""
                sq = scrp.tile([TOK, D_MODEL], F32, tag="sq")
                ssq = tinyp.tile([TOK, 1], F32, tag="ssq")
                nc.scalar.activation(sq[:], h_in[:], AF.Square,
                                     bias=zeps[0:TOK, 0:1], accum_out=ssq[:])
                mn = tinyp.tile([TOK, 1], F32, tag="mn")
                nc.scalar.activation(mn[:], ssq[:], AF.Identity,
                                     scale=1.0 / D_MODEL, bias=zeps[0:TOK, 1:2])
                lnv = tinyp.tile([TOK, 1], F32, tag="lnv")
                nc.scalar.activation(lnv[:], mn[:], AF.Ln, bias=zeps[0:TOK, 0:1])
                s = tinyp.tile([TOK, 1], F32, tag="s")
                nc.scalar.activation(s[:], lnv[:], AF.Exp, scale=-0.5,
                                     bias=zeps[0:TOK, 0:1])
                return s

            def to_fm(h_tm_f32):
                """[TOK, 768] f32 -> list of NKD bf16 tiles [128, TOK] (feature-major)"""
                tiles = []
                for kd in range(NKD):
                    pt = psA.tile([128, TOK], F32, tag="ps")
                    nc.tensor.transpose(pt[:], h_tm_f32[:, 128 * kd:128 * (kd + 1)],
                                        eye[0:TOK, 0:TOK])
                    sb = hnfmp.tile([128, TOK], BF16, tag="hnfm")
                    nc.scalar.activation(sb[:], pt[:], AF.Copy)
                    tiles.append(sb)
                return tiles

            for layer in range(N_LAYERS):
                # ---------------- rmsnorm + feature-major ----------------
                s = rmsnorm_scale(h)
                hn = scrp.tile([TOK, D_MODEL], F32, tag="hn")
                nc.vector.tensor_scalar_mul(hn[:], h[:], s[:])
                hn_fm = to_fm(hn)

                # layer weights
                sm = wsmp.tile([128, NCH, 8], F32, tag="sm")
                nc.sync.dma_start(sm[:], sm_d[layer])
                inw = []
                for kd in range(NKD):
                    wt = wrowp.tile([128, 2 * D_INNER], BF16, tag="wrow")
                    nc.sync.dma_start(wt[:], inT_d[layer, 128 * kd:128 * (kd + 1), :])
                    inw.append(wt)
                outw = []
                xpw = []
                for k in range(NCH):
                    wo = worowp.tile([128, D_MODEL], BF16, tag="worow")
                    nc.sync.dma_start(wo[:], outT_d[layer, 128 * k:128 * (k + 1), :])
                    outw.append(wo)
                    wx = wxpp.tile([128, 112], BF16, tag="wxp")
                    nc.sync.dma_start(wx[:], xpT_d[layer, 128 * k:128 * (k + 1), :])
                    xpw.append(wx)
                wdt = wdtp.tile([DT_RANK, D_INNER], BF16, tag="wdt")
                nc.sync.dma_start(wdt[:], dtT_d[layer])

                # ---------------- in_proj (+conv+silu / silu(z)) ----------
                xc = []      # conv+silu output, bf16 [128, TOK] per chunk
                sz = []      # silu(z) bf16 [128, TOK] per chunk
                proj_ps = psA.tile([112, TOK], F32, tag="ps")
                for m in range(2 * NCH):
                    xz = psA.tile([128, TOK], F32, tag="ps")
                    for kd in range(NKD):
                        nc.tensor.matmul(xz[:], inw[kd][:, 128 * m:128 * (m + 1)],
                                         hn_fm[kd][:], start=(kd == 0),
                                         stop=(kd == NKD - 1))
                    if m < NCH:
                        # x chunk: depthwise causal conv along t
                        k = m
                        xv = xz[:].rearrange("p (b t) -> p b t", b=B_LOC)
                        acc = convp.tile([128, TOK], F32, tag="cacc")
                        accv = acc[:].rearrange("p (b t) -> p b t", b=B_LOC)
                        nc.vector.tensor_scalar_mul(acc[:], xz[:], sm[:, k, 3:4])
                        for kk in (2, 1, 0):
                            sh = 3 - kk
                            nc.vector.scalar_tensor_tensor(
                                accv[:, :, sh:SEQ], xv[:, :, 0:SEQ - sh],
                                sm[:, k, kk:kk + 1], accv[:, :, sh:SEQ],
                                op0=OP.mult, op1=OP.add)
                        xcs = xcp.tile([128, TOK], BF16, tag="xc")
                        nc.scalar.activation(xcs[:], acc[:], AF_SILU,
                                             bias=sm[:, k, 4:5])
                        xc.append(xcs)
                        # x_proj partial accumulation
                        nc.tensor.matmul(proj_ps[:], xpw[k][:], xcs[:],
                                         start=(k == 0), stop=(k == NCH - 1))
                    else:
                        zs = xcp.tile([128, TOK], BF16, tag="sz")
                        nc.scalar.activation(zs[:], xz[:], AF_SILU, bias=zeps[:, 0:1])
                        sz.append(zs)

                # ---------------- x_proj outputs: dt_lo, B, C -------------
                dtlo = bcp.tile([DT_RANK, TOK], BF16, tag="dtlo")
                nc.scalar.activation(dtlo[:], proj_ps[0:DT_RANK, :], AF.Copy)
                bflat = bcp.tile([1, D_STATE * TOK], BF16, tag="bflat")
                cflat = bcp.tile([1, D_STATE * TOK], BF16, tag="cflat")
                bsm = bcp.tile([D_STATE, TOK], BF16, tag="bsm")
                csm = bcp.tile([D_STATE, TOK], BF16, tag="csm")
                nc.scalar.activation(bsm[:], proj_ps[64:80, :], AF.Copy)
                nc.scalar.activation(csm[:], proj_ps[96:112, :], AF.Copy)
                bdr = dramp.tile([D_STATE, TOK], BF16, tag="bdr")
                cdr = dramp.tile([D_STATE, TOK], BF16, tag="cdr")
                nc.sync.dma_start(bdr[:], bsm[:])
                nc.sync.dma_start(cdr[:], csm[:])
                nc.sync.dma_start(bflat[:], bdr[:].rearrange("a b -> (a b)").unsqueeze(0))
                nc.sync.dma_start(cflat[:], cdr[:].rearrange("a b -> (a b)").unsqueeze(0))
                # replicate across partitions via rank-1 matmul
                SV = D_STATE * TOK  # 1024
                brep = bcp.tile([128, SV], BF16, tag="breps")
                crep = bcp.tile([128, SV], BF16, tag="creps")
                for half in range(2):
                    sl = slice(512 * half, 512 * (half + 1))
                    rp = pbig.tile([128, 512], F32, tag="pb")
                    nc.tensor.matmul(rp[:], ones1[:], bflat[:, sl])
                    nc.scalar.activation(brep[:, sl], rp[:], AF.Copy)
                    rp2 = pbig.tile([128, 512], F32, tag="pb")
                    nc.tensor.matmul(rp2[:], ones1[:], cflat[:, sl])
                    nc.scalar.activation(crep[:, sl], rp2[:], AF.Copy)
                # layout of brep free dim is (n, b, t)
                brv = brep[:].rearrange("p (n b t) -> p b n t", n=D_STATE, b=B_LOC)
                crv = crep[:].rearrange("p (n b t) -> p b t n", n=D_STATE, b=B_LOC)

                # ---------------- dt + scan per channel chunk -------------
                hup_ps = psB.tile([TOK, D_MODEL], F32, tag="hup")
                for k in range(NCH):
                    dt_ps = psA.tile([128, TOK], F32, tag="ps")
                    nc.tensor.matmul(dt_ps[:], wdt[:, 128 * k:128 * (k + 1)],
                                     dtlo[:], start=True, stop=True)
                    spe = ktmp.tile([128, TOK], F32, tag="spe")
                    nc.scalar.activation(spe[:], dt_ps[:], AF.Exp,
                                         bias=sm[:, k, 5:6])
                    dt = ktmp.tile([128, TOK], BF16, tag="dt")
                    nc.scalar.activation(dt[:], spe[:], AF.Ln,
                                         bias=zeps[:, 2:3])
                    r = ktmp.tile([128, TOK], BF16, tag="r")
                    nc.scalar.activation(r[:], dt[:], AF.Exp, scale=-1.0, bias=zeps[:, 0:1])
                    # zero the t=0 column of r (scan segment reset)
                    rv = r[:].rearrange("p (b t) -> p b t", b=B_LOC)
                    nc.vector.memset(rv[:, :, 0:1], 0.0)
                    dtx = ktmp.tile([128, TOK], BF16, tag="dtx")
                    nc.vector.tensor_tensor(dtx[:], dt[:], xc[k][:], op=OP.mult)

                    # dA powers: physical layout (b, n, t)
                    dA = scanp.tile([128, B_LOC * D_STATE * SEQ], BF16, tag="dA")
                    dav = dA[:].rearrange("p (b n t) -> p b n t", b=B_LOC,
                                          n=D_STATE)
                    nc.vector.tensor_copy(dav[:, :, 0:1, :],
                                          rv[:, :, :].unsqueeze(2))
                    for pw in range(4):
                        lo, sz_n = (1, 1) if pw == 0 else (2 ** pw, 2 ** pw)
                        # dA[n in lo..lo+sz_n) = dA[n-lo] * dA[lo-1]
                        nc.vector.tensor_tensor(
                            dav[:, :, lo:lo + sz_n, :],
                            dav[:, :, 0:sz_n, :],
                            dav[:, :, lo - 1:lo, :].broadcast_to(
                                [128, B_LOC, sz_n, SEQ]),
                            op=OP.mult)
                    # dBx = dtx * B
                    dBx = scanp.tile([128, B_LOC * D_STATE * SEQ], BF16, tag="dBx")
                    dbv = dBx[:].rearrange("p (b n t) -> p b n t", b=B_LOC,
                                           n=D_STATE)
                    dtxv = dtx[:].rearrange("p (b t) -> p b t", b=B_LOC)
                    nc.vector.tensor_tensor(
                        dbv[:], dtxv[:, :, :].unsqueeze(2).broadcast_to(
                            [128, B_LOC, D_STATE, SEQ]),
                        brv[:], op=OP.mult)
                    # recurrence along t
                    hsc = scanp.tile([128, B_LOC * D_STATE * SEQ], BF16, tag="hsc")
                    nc.vector.tensor_tensor_scan(hsc[:], dA[:], dBx[:], 0.0,
                                                 op0=OP.mult, op1=OP.add)
                    # y = sum_n h * C
                    hCt = scanp.tile([128, B_LOC * D_STATE * SEQ], BF16, tag="hC")
                    hcv = hCt[:].rearrange("p (b n t) -> p b t n", b=B_LOC,
                                           n=D_STATE)
                    hv = hsc[:].rearrange("p (b n t) -> p b t n", b=B_LOC,
                                          n=D_STATE)
                    nc.vector.tensor_tensor(hcv[:], hv[:], crv[:], op=OP.mult)
                    y = ktmp.tile([128, TOK], F32, tag="y")
                    yv = y[:].rearrange("p (b t) -> p b t", b=B_LOC)
                    nc.vector.tensor_reduce(yv[:], hcv[:], axis=mybir.AxisListType.X,
                                            op=OP.add)
                    # y += D * xc ; y *= silu(z)
                    nc.vector.scalar_tensor_tensor(y[:], xc[k][:], sm[:, k, 6:7],
                                                   y[:], op0=OP.mult, op1=OP.add)
                    ym = ktmp.tile([128, TOK], BF16, tag="ym")
                    nc.vector.tensor_tensor(ym[:], y[:], sz[k][:], op=OP.mult)
                    # out_proj partial (N-chunks must stay inside a PSUM bank)
                    for sl in (slice(0, 512), slice(512, 768)):
                        nc.tensor.matmul(hup_ps[:, sl], ym[:], outw[k][:, sl],
                                         start=(k == 0), stop=(k == NCH - 1))

                h_new = hpool.tile([TOK, D_MODEL], F32, tag="h")
                nc.vector.tensor_tensor(h_new[:], h[:], hup_ps[:], op=OP.add)
                h = h_new

            # ---------------- final norm + LM head --------------------
            s = rmsnorm_scale(h)
            hf = scrp.tile([TOK, D_MODEL], F32, tag="hn")
            nc.vector.tensor_scalar_mul(hf[:], h[:], s[:])
            hf_fm = to_fm(hf)

            NV = 512
            nvc = (VOCAB + NV - 1) // NV
            for v in range(nvc):
                v0 = NV * v
                width = min(NV, VOCAB - v0)
                lg = psA.tile([TOK, NV], F32, tag="ps")
                for kd in range(NKD):
                    et = embp.tile([128, NV], BF16, tag="emb")
                    nc.sync.dma_start(et[:, 0:width],
                                      embT_d[128 * kd:128 * (kd + 1),
                                             v0:v0 + width])
                    nc.tensor.matmul(lg[:, 0:width], hf_fm[kd][:],
                                     et[:, 0:width], start=(kd == 0),
                                     stop=(kd == NKD - 1))
                lo = lgoutp.tile([TOK, NV], F32, tag="lgout")
                nc.scalar.activation(lo[:, 0:width], lg[:, 0:width], AF.Copy)
                nc.sync.dma_start(out_d[:, v0:v0 + width], lo[:, 0:width])

    nc.compile()
    return nc


def _prep_weights(embed, norm_w, in_proj_w, conv_w, conv_b, x_proj_w,
                  dt_proj_w, dt_proj_b, A_log, D, out_proj_w, norm_f_w):
    bf = ml_dtypes.bfloat16
    L = N_LAYERS
    # fold rmsnorm gain into in_proj weight
    w_in = in_proj_w[:L] * norm_w[:L][:, None, :]          # [L, 2di, d]
    inT = np.ascontiguousarray(w_in.transpose(0, 2, 1)).astype(bf)
    outT = np.ascontiguousarray(out_proj_w[:L].transpose(0, 2, 1)).astype(bf)
    xpT_raw = x_proj_w[:L].transpose(0, 2, 1)   # [L, d_inner, 80]
    xpT = np.zeros((L, D_INNER, 112), np.float32)
    xpT[:, :, 0:DT_RANK] = xpT_raw[:, :, 0:DT_RANK]
    xpT[:, :, 64:80] = xpT_raw[:, :, DT_RANK:DT_RANK + D_STATE]
    xpT[:, :, 96:112] = xpT_raw[:, :, DT_RANK + D_STATE:]
    xpT = xpT.astype(bf)
    dtT = np.ascontiguousarray(dt_proj_w[:L].transpose(0, 2, 1)).astype(bf)
    embT = np.ascontiguousarray((embed * norm_f_w[None, :]).T).astype(bf)

    sm = np.zeros((L, 128, NCH, 8), np.float32)
    for k in range(NCH):
        sl = slice(128 * k, 128 * (k + 1))
        sm[:, :, k, 0:4] = conv_w[:L, sl, :]
        sm[:, :, k, 4] = conv_b[:L, sl]
        sm[:, :, k, 5] = dt_proj_b[:L, sl]
        sm[:, :, k, 6] = D[:L, sl]

    eye = np.eye(128, dtype=np.float32)
    ones1 = np.ones((1, 128), dtype=bf)
    vtag = np.zeros((1, KERNEL_VERSION * 64 + N_LAYERS), np.float32)
    return dict(inT=inT, outT=outT, xpT=xpT, dtT=dtT, embT=embT,
                smalls=sm, eye=eye, ones1=ones1, vtag=vtag)


def kernel(full_ids, full_mask, full_loss_mask, embed, norm_w, in_proj_w,
           conv_w, conv_b, x_proj_w, dt_proj_w, dt_proj_b, A_log, D,
           out_proj_w, norm_f_w, _return_results=False, _trace=False):
    full_ids = np.asarray(full_ids)
    assert np.all(np.asarray(full_mask)[:, :SEQ] == 1), "kernel assumes mask==1"
    # A_log structure check: A[ch, n] = -(n+1) for all ch (mamba-130m init);
    # the scan uses decay r^n with r = exp(-dt), which requires this.
    A = -np.exp(np.asarray(A_log, np.float64))
    assert np.allclose(A, -np.arange(1, D_STATE + 1)[None, None, :],
                       rtol=1e-5), "kernel requires A[ch,n] = -(n)"

    weights = _prep_weights(embed, norm_w, in_proj_w, conv_w, conv_b,
                            x_proj_w, dt_proj_w, dt_proj_b, A_log, D,
                            out_proj_w, norm_f_w)

    ids = np.asarray(full_ids[:, :SEQ]).astype(np.int64)
    h0_full = np.asarray(embed, np.float32)[ids]       # [64, 8, 768] gather
    in_maps = []
    for c in range(NCORES):
        m = dict(weights)
        m["h0"] = np.ascontiguousarray(
            h0_full[B_LOC * c:B_LOC * (c + 1)].reshape(TOK, D_MODEL))
        in_maps.append(m)

    if "prog" not in _cache:
        _cache["prog"] = _build_program()
    nc = _cache["prog"]
    res = run_bass_kernel_spmd(nc, in_maps, core_ids=list(range(NCORES)),
                               trace=_trace)
    parts = [res.results[c]["logits"].reshape(B_LOC, SEQ, VOCAB)
             for c in range(NCORES)]
    out = np.concatenate(parts, axis=0)
    if _return_results:
        return out, res
    return out


# revision 16
# speedup vs baseline: 14401.8675x; 14401.8675x over previous
"""Mamba-130m forward (B=64, T=8, 24 layers, tied LM head) on 8 Trainium2 cores.

Strategy: data-parallel over batch (8 batches/core, 64 tokens/core), full layer
stack per core, token-sharded LM head (each core computes the full vocab for its
own tokens).  No collectives.  Weights are pre-transposed / folded on the host
and replicated to every core in bf16; all matmuls run bf16 with fp32 PSUM
accumulation.  The selective scan runs on the Vector engine with
tensor_tensor_scan, exploiting A[ch, n] = -n (verified at runtime) so the decay
factors are powers r^n of r = exp(-dt).
"""

import os
import sys
import tempfile

# The libneuronxla NEFF cache hashes the HLO wrapper but not the embedded BIR
# (frontend attributes), so two different Bass programs collide on the same
# cache key.  Use a fresh per-process cache dir to avoid stale-NEFF reuse.
if "NEURON_COMPILE_CACHE_URL" not in os.environ:
    os.environ["NEURON_COMPILE_CACHE_URL"] = tempfile.mkdtemp(
        prefix="neuron-cache-")

sys.path.insert(0, "/opt/trn_rl_repo")

import numpy as np
import ml_dtypes

import concourse.bass as bass
import concourse.bacc as bacc
import concourse.tile as tile
import concourse.mybir as mybir
from concourse.bass_utils import run_bass_kernel_spmd
from concourse.tile_rust import add_dep_helper

F32 = mybir.dt.float32
BF16 = mybir.dt.bfloat16
AF = mybir.ActivationFunctionType
OP = mybir.AluOpType
# debug: CoreSim has no Silu; MK_SIGMOID=1 swaps in Sigmoid (wrong math, same
# program structure) for sim-vs-hw divergence checks.
AF_SILU = AF.Sigmoid if os.environ.get("MK_SIGMOID", "0") == "1" else AF.Silu

N_LAYERS = int(os.environ.get("MK_LAYERS", "24"))
# Bump on every kernel change: encoded into a dummy input's shape so the HLO
# hash (and the terminal-side NEFF cache key) changes with the program.
KERNEL_VERSION = 4
D_MODEL = 768
D_INNER = 1536
D_STATE = 16
DT_RANK = 48
CONV_K = 4
VOCAB = 50280
EPS = 1e-5
SEQ = 8
B_FULL = 64
NCORES = 8
B_LOC = B_FULL // NCORES      # 8 batches per core
TOK = B_LOC * SEQ             # 64 tokens per core, tok = b*8 + t
NKD = D_MODEL // 128          # 6 d_model chunks
NCH = D_INNER // 128          # 12 d_inner chunks

_cache = {}


def _build_program():
    """Build the SPMD Bass program (identical on all 8 cores)."""
    nc = bacc.Bacc("TRN2", target_bir_lowering=False, debug=False,
                   num_devices=NCORES)

    # ---- DRAM I/O ----
    h0_d = nc.dram_tensor("h0", [TOK, D_MODEL], F32, kind="ExternalInput")
    inT_d = nc.dram_tensor("inT", [N_LAYERS, D_MODEL, 2 * D_INNER], BF16,
                           kind="ExternalInput")
    outT_d = nc.dram_tensor("outT", [N_LAYERS, D_INNER, D_MODEL], BF16,
                            kind="ExternalInput")
    xpT_d = nc.dram_tensor("xpT", [N_LAYERS, D_INNER, 112],
                           BF16, kind="ExternalInput")
    dtT_d = nc.dram_tensor("dtT", [N_LAYERS, DT_RANK, D_INNER], BF16,
                           kind="ExternalInput")
    embT_d = nc.dram_tensor("embT", [D_MODEL, VOCAB], BF16,
                            kind="ExternalInput")
    # per-layer per-chunk scalars: [L, 128, NCH, 8] = conv_w(4), conv_b, dt_b, D, pad
    sm_d = nc.dram_tensor("smalls", [N_LAYERS, 128, NCH, 8], F32,
                          kind="ExternalInput")
    eye_d = nc.dram_tensor("eye", [128, 128], F32, kind="ExternalInput")
    ones_d = nc.dram_tensor("ones1", [1, 128], BF16, kind="ExternalInput")
    nc.dram_tensor("vtag", [1, KERNEL_VERSION * 64 + N_LAYERS], F32,
                   kind="ExternalInput")
    out_d = nc.dram_tensor("logits", [TOK, VOCAB], F32, kind="ExternalOutput")

    from contextlib import ExitStack
    with ExitStack() as ctx:
        tc = ctx.enter_context(tile.TileContext(nc))
        constp = ctx.enter_context(tc.tile_pool(name="const", bufs=1))
        hpool = ctx.enter_context(tc.tile_pool(name="hpool", bufs=2))
        scrp = ctx.enter_context(tc.tile_pool(name="scr", bufs=2))
        tinyp = ctx.enter_context(tc.tile_pool(name="tiny", bufs=4))
        hnfmp = ctx.enter_context(tc.tile_pool(name="hnfm", bufs=14))
        wrowp = ctx.enter_context(tc.tile_pool(name="wrow", bufs=7))
        worowp = ctx.enter_context(tc.tile_pool(name="worow", bufs=14))
        wxpp = ctx.enter_context(tc.tile_pool(name="wxp", bufs=14))
        wdtp = ctx.enter_context(tc.tile_pool(name="wdt", bufs=2))
        wsmp = ctx.enter_context(tc.tile_pool(name="wsm", bufs=2))
        xcp = ctx.enter_context(tc.tile_pool(name="xcp", bufs=14))
        convp = ctx.enter_context(tc.tile_pool(name="convp", bufs=3))
        ktmp = ctx.enter_context(tc.tile_pool(name="ktmp", bufs=4))
        scanp = ctx.enter_context(tc.tile_pool(name="scan", bufs=3))
        bcp = ctx.enter_context(tc.tile_pool(name="bc", bufs=2))
        embp = ctx.enter_context(tc.tile_pool(name="emb", bufs=10))
        lgoutp = ctx.enter_context(tc.tile_pool(name="lgout", bufs=6))
        psA = ctx.enter_context(tc.tile_pool(name="psA", bufs=3, space="PSUM"))
        pbig = ctx.enter_context(tc.tile_pool(name="pbig", bufs=2, space="PSUM"))
        psB = ctx.enter_context(tc.tile_pool(name="psB", bufs=1, space="PSUM"))
        dramp = ctx.enter_context(tc.tile_pool(name="dram", bufs=4, space="DRAM"))
        if True:
            eye = constp.tile([128, 128], F32)
            nc.sync.dma_start(eye[:], eye_d[:, :])
            ones1 = constp.tile([1, 128], BF16)
            nc.sync.dma_start(ones1[:], ones_d[:, :])
            zeps = constp.tile([128, 3], F32)
            nc.vector.memset(zeps[:, 0:1], 0.0)
            nc.vector.memset(zeps[:, 1:2], EPS)
            nc.vector.memset(zeps[:, 2:3], 1.0)

            h = hpool.tile([TOK, D_MODEL], F32, tag="h")
            nc.sync.dma_start(h[:], h0_d[:, :])

            def rmsnorm_scale(h_in):
                """returns s [TOK,1] f32 with s = rsqrt(mean(h^2)+eps)"""
                sq = scrp.tile([TOK, D_MODEL], F32, tag="sq")
                ssq = tinyp.tile([TOK, 1], F32, tag="ssq")
                nc.scalar.activation(sq[:], h_in[:], AF.Square,
                                     bias=zeps[0:TOK, 0:1], accum_out=ssq[:])
                mn = tinyp.tile([TOK, 1], F32, tag="mn")
                nc.scalar.activation(mn[:], ssq[:], AF.Identity,
                                     scale=1.0 / D_MODEL, bias=zeps[0:TOK, 1:2])
                lnv = tinyp.tile([TOK, 1], F32, tag="lnv")
                nc.scalar.activation(lnv[:], mn[:], AF.Ln, bias=zeps[0:TOK, 0:1])
                s = tinyp.tile([TOK, 1], F32, tag="s")
                nc.scalar.activation(s[:], lnv[:], AF.Exp, scale=-0.5,
                                     bias=zeps[0:TOK, 0:1])
                return s

            def to_fm(h_tm_f32):
                """[TOK, 768] f32 -> list of NKD bf16 tiles [128, TOK] (feature-major)"""
                tiles = []
                for kd in range(NKD):
                    pt = psA.tile([128, TOK], F32, tag="ps")
                    nc.tensor.transpose(pt[:], h_tm_f32[:, 128 * kd:128 * (kd + 1)],
                                        eye[0:TOK, 0:TOK])
                    sb = hnfmp.tile([128, TOK], BF16, tag="hnfm")
                    nc.scalar.activation(sb[:], pt[:], AF.Copy)
                    tiles.append(sb)
                return tiles

            for layer in range(N_LAYERS):
                # ---------------- rmsnorm + feature-major ----------------
                s = rmsnorm_scale(h)
                hn = scrp.tile([TOK, D_MODEL], F32, tag="hn")
                nc.vector.tensor_scalar_mul(hn[:], h[:], s[:])
                hn_fm = to_fm(hn)

                # layer weights
                sm = wsmp.tile([128, NCH, 8], F32, tag="sm")
                nc.sync.dma_start(sm[:], sm_d[layer])
                inw = []
                for kd in range(NKD):
                    wt = wrowp.tile([128, 2 * D_INNER], BF16, tag="wrow")
                    nc.sync.dma_start(wt[:], inT_d[layer, 128 * kd:128 * (kd + 1), :])
                    inw.append(wt)
                outw = []
                xpw = []
                for k in range(NCH):
                    wo = worowp.tile([128, D_MODEL], BF16, tag="worow")
                    nc.sync.dma_start(wo[:], outT_d[layer, 128 * k:128 * (k + 1), :])
                    outw.append(wo)
                    wx = wxpp.tile([128, 112], BF16, tag="wxp")
                    nc.sync.dma_start(wx[:], xpT_d[layer, 128 * k:128 * (k + 1), :])
                    xpw.append(wx)
                wdt = wdtp.tile([DT_RANK, D_INNER], BF16, tag="wdt")
                nc.sync.dma_start(wdt[:], dtT_d[layer])

                # ---------------- in_proj (+conv+silu / silu(z)) ----------
                xc = []      # conv+silu output, bf16 [128, TOK] per chunk
                sz = []      # silu(z) bf16 [128, TOK] per chunk
                silu_insts = []
                proj_ps = psA.tile([112, TOK], F32, tag="ps")
                for m in range(2 * NCH):
                    xz = psA.tile([128, TOK], F32, tag="ps")
                    for kd in range(NKD):
                        nc.tensor.matmul(xz[:], inw[kd][:, 128 * m:128 * (m + 1)],
                                         hn_fm[kd][:], start=(kd == 0),
                                         stop=(kd == NKD - 1))
                    if m < NCH:
                        # x chunk: depthwise causal conv along t
                        k = m
                        xv = xz[:].rearrange("p (b t) -> p b t", b=B_LOC)
                        acc = convp.tile([128, TOK], F32, tag="cacc")
                        accv = acc[:].rearrange("p (b t) -> p b t", b=B_LOC)
                        nc.vector.tensor_scalar_mul(acc[:], xz[:], sm[:, k, 3:4])
                        for kk in (2, 1, 0):
                            sh = 3 - kk
                            nc.vector.scalar_tensor_tensor(
                                accv[:, :, sh:SEQ], xv[:, :, 0:SEQ - sh],
                                sm[:, k, kk:kk + 1], accv[:, :, sh:SEQ],
                                op0=OP.mult, op1=OP.add)
                        xcs = xcp.tile([128, TOK], BF16, tag="xc")
                        silu_insts.append(nc.scalar.activation(
                            xcs[:], acc[:], AF_SILU, bias=sm[:, k, 4:5]))
                        xc.append(xcs)
                        # x_proj partial accumulation
                        nc.tensor.matmul(proj_ps[:], xpw[k][:], xcs[:],
                                         start=(k == 0), stop=(k == NCH - 1))
                    else:
                        zs = xcp.tile([128, TOK], BF16, tag="sz")
                        silu_insts.append(nc.scalar.activation(
                            zs[:], xz[:], AF_SILU, bias=zeps[:, 0:1]))
                        sz.append(zs)

                # ---------------- x_proj outputs: dt_lo, B, C -------------
                dtlo = bcp.tile([DT_RANK, TOK], BF16, tag="dtlo")
                nc.scalar.activation(dtlo[:], proj_ps[0:DT_RANK, :], AF.Copy)
                bflat = bcp.tile([1, D_STATE * TOK], BF16, tag="bflat")
                cflat = bcp.tile([1, D_STATE * TOK], BF16, tag="cflat")
                bsm = bcp.tile([D_STATE, TOK], BF16, tag="bsm")
                csm = bcp.tile([D_STATE, TOK], BF16, tag="csm")
                nc.scalar.activation(bsm[:], proj_ps[64:80, :], AF.Copy)
                nc.scalar.activation(csm[:], proj_ps[96:112, :], AF.Copy)
                bdr = dramp.tile([D_STATE, TOK], BF16, tag="bdr")
                cdr = dramp.tile([D_STATE, TOK], BF16, tag="cdr")
                nc.sync.dma_start(bdr[:], bsm[:])
                nc.sync.dma_start(cdr[:], csm[:])
                nc.sync.dma_start(bflat[:], bdr[:].rearrange("a b -> (a b)").unsqueeze(0))
                nc.sync.dma_start(cflat[:], cdr[:].rearrange("a b -> (a b)").unsqueeze(0))
                # replicate across partitions via rank-1 matmul
                SV = D_STATE * TOK  # 1024
                brep = bcp.tile([128, SV], BF16, tag="breps")
                crep = bcp.tile([128, SV], BF16, tag="creps")
                for half in range(2):
                    sl = slice(512 * half, 512 * (half + 1))
                    rp = pbig.tile([128, 512], F32, tag="pb")
                    nc.tensor.matmul(rp[:], ones1[:], bflat[:, sl])
                    nc.scalar.activation(brep[:, sl], rp[:], AF.Copy)
                    rp2 = pbig.tile([128, 512], F32, tag="pb")
                    nc.tensor.matmul(rp2[:], ones1[:], cflat[:, sl])
                    nc.scalar.activation(crep[:, sl], rp2[:], AF.Copy)
                # layout of brep free dim is (n, b, t)
                brv = brep[:].rearrange("p (n b t) -> p b n t", n=D_STATE, b=B_LOC)
                crv = crep[:].rearrange("p (n b t) -> p b t n", n=D_STATE, b=B_LOC)

                # ---------------- dt + scan per channel chunk -------------
                hup_ps = psB.tile([TOK, D_MODEL], F32, tag="hup")
                for k in range(NCH):
                    dt_ps = psA.tile([128, TOK], F32, tag="ps")
                    nc.tensor.matmul(dt_ps[:], wdt[:, 128 * k:128 * (k + 1)],
                                     dtlo[:], start=True, stop=True)
                    spe = ktmp.tile([128, TOK], F32, tag="spe")
                    nc.scalar.activation(spe[:], dt_ps[:], AF.Exp,
                                         bias=sm[:, k, 5:6])
                    dt = ktmp.tile([128, TOK], BF16, tag="dt")
                    nc.scalar.activation(dt[:], spe[:], AF.Ln,
                                         bias=zeps[:, 2:3])
                    r = ktmp.tile([128, TOK], BF16, tag="r")
                    nc.scalar.activation(r[:], dt[:], AF.Exp, scale=-1.0, bias=zeps[:, 0:1])
                    # zero the t=0 column of r (scan segment reset)
                    rv = r[:].rearrange("p (b t) -> p b t", b=B_LOC)
                    nc.vector.memset(rv[:, :, 0:1], 0.0)
                    dtx = ktmp.tile([128, TOK], BF16, tag="dtx")
                    nc.vector.tensor_tensor(dtx[:], dt[:], xc[k][:], op=OP.mult)

                    # dA powers: physical layout (b, n, t)
                    dA = scanp.tile([128, B_LOC * D_STATE * SEQ], BF16, tag="dA")
                    dav = dA[:].rearrange("p (b n t) -> p b n t", b=B_LOC,
                                          n=D_STATE)
                    nc.vector.tensor_copy(dav[:, :, 0:1, :],
                                          rv[:, :, :].unsqueeze(2))
                    for pw in range(4):
                        lo, sz_n = (1, 1) if pw == 0 else (2 ** pw, 2 ** pw)
                        # dA[n in lo..lo+sz_n) = dA[n-lo] * dA[lo-1]
                        nc.vector.tensor_tensor(
                            dav[:, :, lo:lo + sz_n, :],
                            dav[:, :, 0:sz_n, :],
                            dav[:, :, lo - 1:lo, :].broadcast_to(
                                [128, B_LOC, sz_n, SEQ]),
                            op=OP.mult)
                    # dBx = dtx * B
                    dBx = scanp.tile([128, B_LOC * D_STATE * SEQ], BF16, tag="dBx")
                    dbv = dBx[:].rearrange("p (b n t) -> p b n t", b=B_LOC,
                                           n=D_STATE)
                    dtxv = dtx[:].rearrange("p (b t) -> p b t", b=B_LOC)
                    nc.vector.tensor_tensor(
                        dbv[:], dtxv[:, :, :].unsqueeze(2).broadcast_to(
                            [128, B_LOC, D_STATE, SEQ]),
                        brv[:], op=OP.mult)
                    # recurrence along t
                    hsc = scanp.tile([128, B_LOC * D_STATE * SEQ], BF16, tag="hsc")
                    nc.vector.tensor_tensor_scan(hsc[:], dA[:], dBx[:], 0.0,
                                                 op0=OP.mult, op1=OP.add)
                    # y = sum_n h * C
                    hCt = scanp.tile([128, B_LOC * D_STATE * SEQ], BF16, tag="hC")
                    hcv = hCt[:].rearrange("p (b n t) -> p b t n", b=B_LOC,
                                           n=D_STATE)
                    hv = hsc[:].rearrange("p (b n t) -> p b t n", b=B_LOC,
                                          n=D_STATE)
                    nc.vector.tensor_tensor(hcv[:], hv[:], crv[:], op=OP.mult)
                    y = ktmp.tile([128, TOK], F32, tag="y")
                    yv = y[:].rearrange("p (b t) -> p b t", b=B_LOC)
                    nc.vector.tensor_reduce(yv[:], hcv[:], axis=mybir.AxisListType.X,
                                            op=OP.add)
                    # y += D * xc ; y *= silu(z)
                    nc.vector.scalar_tensor_tensor(y[:], xc[k][:], sm[:, k, 6:7],
                                                   y[:], op0=OP.mult, op1=OP.add)
                    ym = ktmp.tile([128, TOK], BF16, tag="ym")
                    nc.vector.tensor_tensor(ym[:], y[:], sz[k][:], op=OP.mult)
                    # out_proj partial (N-chunks must stay inside a PSUM bank)
                    for sl in (slice(0, 512), slice(512, 768)):
                        nc.tensor.matmul(hup_ps[:, sl], ym[:], outw[k][:, sl],
                                         start=(k == 0), stop=(k == NCH - 1))

                h_new = hpool.tile([TOK, D_MODEL], F32, tag="h")
                nc.vector.tensor_tensor(h_new[:], h[:], hup_ps[:], op=OP.add)
                h = h_new

            # ---------------- final norm + LM head --------------------
            s = rmsnorm_scale(h)
            hf = scrp.tile([TOK, D_MODEL], F32, tag="hn")
            nc.vector.tensor_scalar_mul(hf[:], h[:], s[:])
            hf_fm = to_fm(hf)

            NV = 512
            nvc = (VOCAB + NV - 1) // NV
            for v in range(nvc):
                v0 = NV * v
                width = min(NV, VOCAB - v0)
                lg = psA.tile([TOK, NV], F32, tag="ps")
                for kd in range(NKD):
                    et = embp.tile([128, NV], BF16, tag="emb")
                    nc.sync.dma_start(et[:, 0:width],
                                      embT_d[128 * kd:128 * (kd + 1),
                                             v0:v0 + width])
                    nc.tensor.matmul(lg[:, 0:width], hf_fm[kd][:],
                                     et[:, 0:width], start=(kd == 0),
                                     stop=(kd == NKD - 1))
                lo = lgoutp.tile([TOK, NV], F32, tag="lgout")
                nc.scalar.activation(lo[:, 0:width], lg[:, 0:width], AF.Copy)
                nc.sync.dma_start(out_d[:, v0:v0 + width], lo[:, 0:width])

    nc.compile()
    return nc


def _prep_weights(embed, norm_w, in_proj_w, conv_w, conv_b, x_proj_w,
                  dt_proj_w, dt_proj_b, A_log, D, out_proj_w, norm_f_w):
    bf = ml_dtypes.bfloat16
    L = N_LAYERS
    # fold rmsnorm gain into in_proj weight
    w_in = in_proj_w[:L] * norm_w[:L][:, None, :]          # [L, 2di, d]
    inT = np.ascontiguousarray(w_in.transpose(0, 2, 1)).astype(bf)
    outT = np.ascontiguousarray(out_proj_w[:L].transpose(0, 2, 1)).astype(bf)
    xpT_raw = x_proj_w[:L].transpose(0, 2, 1)   # [L, d_inner, 80]
    xpT = np.zeros((L, D_INNER, 112), np.float32)
    xpT[:, :, 0:DT_RANK] = xpT_raw[:, :, 0:DT_RANK]
    xpT[:, :, 64:80] = xpT_raw[:, :, DT_RANK:DT_RANK + D_STATE]
    xpT[:, :, 96:112] = xpT_raw[:, :, DT_RANK + D_STATE:]
    xpT = xpT.astype(bf)
    dtT = np.ascontiguousarray(dt_proj_w[:L].transpose(0, 2, 1)).astype(bf)
    embT = np.ascontiguousarray((embed * norm_f_w[None, :]).T).astype(bf)

    sm = np.zeros((L, 128, NCH, 8), np.float32)
    for k in range(NCH):
        sl = slice(128 * k, 128 * (k + 1))
        sm[:, :, k, 0:4] = conv_w[:L, sl, :]
        sm[:, :, k, 4] = conv_b[:L, sl]
        sm[:, :, k, 5] = dt_proj_b[:L, sl]
        sm[:, :, k, 6] = D[:L, sl]

    eye = np.eye(128, dtype=np.float32)
    ones1 = np.ones((1, 128), dtype=bf)
    vtag = np.zeros((1, KERNEL_VERSION * 64 + N_LAYERS), np.float32)
    return dict(inT=inT, outT=outT, xpT=xpT, dtT=dtT, embT=embT,
                smalls=sm, eye=eye, ones1=ones1, vtag=vtag)


def kernel(full_ids, full_mask, full_loss_mask, embed, norm_w, in_proj_w,
           conv_w, conv_b, x_proj_w, dt_proj_w, dt_proj_b, A_log, D,
           out_proj_w, norm_f_w, _return_results=False, _trace=False):
    full_ids = np.asarray(full_ids)
    assert np.all(np.asarray(full_mask)[:, :SEQ] == 1), "kernel assumes mask==1"
    # A_log structure check: A[ch, n] = -(n+1) for all ch (mamba-130m init);
    # the scan uses decay r^n with r = exp(-dt), which requires this.
    A = -np.exp(np.asarray(A_log, np.float64))
    assert np.allclose(A, -np.arange(1, D_STATE + 1)[None, None, :],
                       rtol=1e-5), "kernel requires A[ch,n] = -(n)"

    weights = _prep_weights(embed, norm_w, in_proj_w, conv_w, conv_b,
                            x_proj_w, dt_proj_w, dt_proj_b, A_log, D,
                            out_proj_w, norm_f_w)

    ids = np.asarray(full_ids[:, :SEQ]).astype(np.int64)
    h0_full = np.asarray(embed, np.float32)[ids]       # [64, 8, 768] gather
    in_maps = []
    for c in range(NCORES):
        m = dict(weights)
        m["h0"] = np.ascontiguousarray(
            h0_full[B_LOC * c:B_LOC * (c + 1)].reshape(TOK, D_MODEL))
        in_maps.append(m)

    if "prog" not in _cache:
        _cache["prog"] = _build_program()
    nc = _cache["prog"]
    res = run_bass_kernel_spmd(nc, in_maps, core_ids=list(range(NCORES)),
                               trace=_trace)
    parts = [res.results[c]["logits"].reshape(B_LOC, SEQ, VOCAB)
             for c in range(NCORES)]
    out = np.concatenate(parts, axis=0)
    if _return_results:
        return out, res
    return out
